# revision 1
# baseline (speedup 1.0000x reference)
"""StyleGAN2 fused upsample2x + 3x3 conv + FIR(1,3,3,1) + bias — TRN2 Bass kernel.

Math: zero-insert-by-2 -> corr(w, full pad) -> pad1 -> corr(FIR 4x4) composes
into a single stride-2 transposed conv with a 6x6 kernel W6 = fir (*) w.
By output parity (alpha, beta) in {0,1}^2 this splits into 4 ordinary 3x3
SAME convs over the original 64x64 input:

  out[n, o, 2u+a, 2v+b] = sum_{c,dr,dc} K[a,b][o,c,dr,dc] * x[n,c,u+dr,v+dc]
  K[a,b][...,di,dj] = W6[..., e_a[di], f_b[dj]],  e_0=(1,3,5), e_1=(0,2,4)

Each parity conv is 9 shifted matmuls (K=256 split in 2x128) accumulated in
PSUM; o=256 is split in 2x128 (M); spatial 64x64 is tiled as 8 chunks of
8 rows x 64 cols (N=512).  Data-parallel over batch: 2 images per core,
8 cores.  Matmuls run in float32r (fast fp32 mode, 1 cycle/row).
"""

import sys

sys.path.insert(0, "/opt/trn_rl_repo")

import numpy as np

import concourse.bacc as bacc
import concourse.mybir as mybir
import concourse.tile as tile
from concourse.bass_utils import run_bass_kernel_spmd

N_CORES = 8
IMGS = 16
IMG_PER_CORE = IMGS // N_CORES  # 2
C = 256  # in channels
O = 256  # out channels
H = W = 64
NK = C // 128  # 2 contraction splits
NM = O // 128  # 2 output-partition splits
NUB = 8  # row-blocks of 8 output (parity-plane) rows
ROWS_PER_UB = 8
HP = H + 2  # padded rows/cols

_compiled = None
LAST_RESULTS = None


def _build():
    nc = bacc.Bacc(None, target_bir_lowering=False, debug=False)
    dt = mybir.dt

    xp_d = nc.dram_tensor(
        "xp", (IMG_PER_CORE, NK, 128, HP * HP), dt.float32r, kind="ExternalInput"
    )
    wt_d = nc.dram_tensor(
        "wt", (128, 4 * 9 * NK * NM * 128), dt.float32r, kind="ExternalInput"
    )
    b_d = nc.dram_tensor("bias", (128, NM), dt.float32, kind="ExternalInput")
    out_d = nc.dram_tensor(
        "out", (IMG_PER_CORE, O, 2 * H, 2 * W), dt.float32, kind="ExternalOutput"
    )

    with tile.TileContext(nc) as tc:
        with (
            tc.tile_pool(name="xpool", bufs=1) as xpool,
            tc.tile_pool(name="wpool", bufs=1) as wpool,
            tc.tile_pool(name="opool", bufs=3) as opool,
            tc.tile_pool(name="psum", bufs=8, space="PSUM") as psum_pool,
        ):
            wt_t = wpool.tile([128, 4 * 9 * NK * NM * 128], dt.float32r, tag="wt")
            xp_t = {}

            def load_xp(img, k, split=False):
                t = xpool.tile([128, HP, HP], dt.float32r, tag=f"xp{img}{k}")
                src = xp_d.ap()[img, k].rearrange("p (h w) -> p h w", h=HP)
                if split:
                    nc.sync.dma_start(t[:, :24, :], src[:, :24, :])
                    nc.sync.dma_start(t[:, 24:, :], src[:, 24:, :])
                else:
                    nc.sync.dma_start(t[:], src)
                xp_t[img, k] = t

            def load_wt(m, par, ks=(0, 1)):
                # weight layout: [m, par, k, tap, o]
                KC = 9 * 128
                base = (m * 4 + par) * NK * KC
                for k in ks:
                    blk = base + k * KC
                    nc.sync.dma_start(
                        wt_t[:, blk : blk + KC], wt_d.ap()[:, blk : blk + KC]
                    )

            # Order: minimal working set first (k-outer accumulation means the
            # first 9 matmuls touch only xp[0,0] top rows + wt chunk (0,0,k0)).
            load_xp(0, 0, split=True)
            load_wt(0, 0, ks=(0,))
            b_t = wpool.tile([128, NM], dt.float32, tag="bias")
            nc.sync.dma_start(b_t[:], b_d.ap()[:])
            load_xp(0, 1)
            load_wt(0, 0, ks=(1,))
            for par in range(1, 4):
                load_wt(0, par)
            for par in range(4):
                load_wt(1, par)
            load_xp(1, 0)
            load_xp(1, 1)

            for img in range(IMG_PER_CORE):
                for m in range(NM):
                    for ub in range(NUB):
                        u0 = ub * ROWS_PER_UB
                        out_sb = opool.tile([128, 2 * ROWS_PER_UB, 2 * W], dt.float32)
                        out_v = out_sb[:].rearrange(
                            "p (u a) (v b) -> p u a v b", a=2, b=2
                        )
                        for a in range(2):
                            for b in range(2):
                                par = a * 2 + b
                                acc = psum_pool.tile(
                                    [128, ROWS_PER_UB, W], dt.float32
                                )
                                n_acc = 0
                                for k in range(NK):
                                    for di in range(3):
                                        for dj in range(3):
                                            tap = di * 3 + dj
                                            idx = ((m * 4 + par) * NK + k) * 9 + tap
                                            rhs = xp_t[img, k][
                                                :,
                                                u0 + di : u0 + di + ROWS_PER_UB,
                                                dj : dj + W,
                                            ]
                                            nc.tensor.matmul(
                                                acc[:],
                                                wt_t[:, idx * 128 : (idx + 1) * 128],
                                                rhs,
                                                start=(n_acc == 0),
                                                stop=(n_acc == NK * 9 - 1),
                                            )
                                            n_acc += 1
                                nc.scalar.activation(
                                    out_v[:, :, a, :, b],
                                    acc[:],
                                    mybir.ActivationFunctionType.Identity,
                                    bias=b_t[:, m : m + 1],
                                )
                        nc.sync.dma_start(
                            out_d.ap()[
                                img,
                                m * 128 : (m + 1) * 128,
                                2 * u0 : 2 * u0 + 2 * ROWS_PER_UB,
                                :,
                            ],
                            out_sb[:],
                        )

    nc.compile()
    return nc


def _compose_weights(w):
    """w (256,256,3,3) -> Wt (128, 4*9*2*2*128) f32, layout
    [c_local, (par, tap, k, m, o_local)]."""
    k1 = np.array([1.0, 3.0, 3.0, 1.0], dtype=np.float64)
    fir = np.outer(k1, k1) / 16.0
    w64 = w.astype(np.float64)
    W6 = np.zeros((O, C, 6, 6), dtype=np.float64)
    for s in range(4):
        for t in range(4):
            W6[:, :, s : s + 3, t : t + 3] += fir[s, t] * w64
    es = [(1, 3, 5), (0, 2, 4)]
    # K_all[a, b, di, dj, k, m, c_local, o_local]
    K_all = np.empty((2, 2, 3, 3, NK, NM, 128, 128), dtype=np.float32)
    for a in range(2):
        for b in range(2):
            for di in range(3):
                for dj in range(3):
                    sub = W6[:, :, es[a][di], es[b][dj]]  # (o, c)
                    for k in range(NK):
                        for m in range(NM):
                            K_all[a, b, di, dj, k, m] = (
                                sub[m * 128 : (m + 1) * 128, k * 128 : (k + 1) * 128]
                                .T.astype(np.float32)
                            )
    # -> [c_local, m, a, b, k, di, dj, o_local]
    return np.ascontiguousarray(K_all.transpose(6, 5, 0, 1, 4, 2, 3, 7)).reshape(
        128, -1
    )


def kernel(x, w, b):
    global _compiled, LAST_RESULTS
    if _compiled is None:
        _compiled = _build()
    nc = _compiled

    x = np.asarray(x, dtype=np.float32)
    w = np.asarray(w, dtype=np.float32)
    b = np.asarray(b, dtype=np.float32)

    wt = _compose_weights(w)
    b2 = np.ascontiguousarray(b.reshape(NM, 128).T)  # [o_local, m]
    xp = np.pad(x, ((0, 0), (0, 0), (1, 1), (1, 1)))  # (16, 256, 66, 66)
    xp = np.ascontiguousarray(
        xp.reshape(N_CORES, IMG_PER_CORE, NK, 128, HP * HP)
    )

    in_maps = [
        {"xp": xp[core], "wt": wt, "bias": b2} for core in range(N_CORES)
    ]
    try:
        res = run_bass_kernel_spmd(nc, in_maps, list(range(N_CORES)))
    except ModuleNotFoundError:
        # BASS_TRACE set in an env without the axon NTFF hook module —
        # retry with tracing disabled.
        import os

        os.environ["BASS_NEVER_TRACE"] = "1"
        res = run_bass_kernel_spmd(nc, in_maps, list(range(N_CORES)))
    LAST_RESULTS = res
    out = np.concatenate([res.results[i]["out"] for i in range(N_CORES)], axis=0)
    return out



# revision 4
# speedup vs baseline: 1.5986x; 1.5986x over previous
"""StyleGAN2 fused upsample2x + 3x3 conv + FIR(1,3,3,1) + bias — TRN2 Bass kernel.

v2 — split the separable FIR: the ROW pass is folded into the conv weights,
the COLUMN pass runs on the vector engines.  Math:

  y1 = corr(zero-dilate-2(x), w, full pad)            # (129,129) per channel
  z[r,c'] = sum_s g[s] y1[r-1+s, c']   g=(1,3,3,1)/4  # row FIR -> 128 rows
  out[R,C] = sum_t g[t] z[R, C-1+t] + bias            # col FIR -> 128 cols

Folding the row FIR into w gives Wr (6 rows x 3 cols).  With output row
parity A = r%2 only 3 row taps survive (p in (1,3,5) for A=0, (0,2,4) for
A=1, all at u-offsets (-1,0,1)); with column parity B = c'%2 the zero
dilation leaves 2 col taps (B=0) or 1 (B=1): 18 matmul taps per output
pixel instead of 36 — half the PE work of composing the full 6x6 kernel.

The 4-tap column FIR is symmetric, so with the 1/4 gain folded into the
weights it is 3 fused element-wise ops: s1 = z[C]+z[C+3], s2 = z[C+1]+z[C+2],
out = 3*s2 + s1 — alternated between DVE and Pool so neither is the
bottleneck.  Bias is folded as delta = bias/8 added to every stored z column
(including the two structural-zero pad columns), since the integer col taps
(1,3,3,1) sum to 8.

Tiling: o=256 in 2x128 (M), contraction 256 in 2x128 (K), spatial as 16
blocks of 4 U-rows: psum tiles [128, 4, 65|64] (260|256 free, >=256 keeps
float32r at 1 cycle/row).  Data-parallel over batch: 2 images/core, 8 cores.
"""

import sys

sys.path.insert(0, "/opt/trn_rl_repo")

import numpy as np

import concourse.bacc as bacc
import concourse.mybir as mybir
import concourse.tile as tile
from concourse.bass_utils import run_bass_kernel_spmd

N_CORES = 8
IMGS = 16
IMG_PER_CORE = IMGS // N_CORES  # 2
C = 256  # in channels
O = 256  # out channels
H = W = 64
NK = C // 128  # 2 contraction splits
NM = O // 128  # 2 output-partition splits
HPR = H + 2  # 66: padded input rows (1 halo each side)
WPC = H + 3  # 67: padded input cols (1 left, 2 right: fp32r needs even free dims)
UB = 16  # row blocks
UROWS = 4  # U rows per block (8 output rows)
ZW = 132  # stored z width: cs=0..130 used (cs=c'+1, pads at 0 and 130)
NBLK = NM * 18 * NK  # 72 weight blocks of 128x128

PL = [(1, 3, 5), (0, 2, 4)]  # Wr row index by (A, ti)
JL = [(0, 2), (1,)]  # Wr col index by (B, tj)
DU = (-1, 0, 1)  # x row offset by ti (same for both A)
DV = [(-1, 0), (0,)]  # x col offset by (B, tj)
NCOL = (66, 64)  # z cols per (B); B=0 col 65 is pad-driven slack (cs=131)


def _blk(m, A, B, k, ti, tj):
    base = m * 36 + A * 18 + (12 if B else 0)
    return base + (k * 3 + ti) * (2 - B) + tj


_compiled = None
LAST_RESULTS = None


def _build():
    nc = bacc.Bacc(None, target_bir_lowering=False, debug=False)
    dt = mybir.dt

    xp_d = nc.dram_tensor(
        "xp", (IMG_PER_CORE, NK, 128, HPR * WPC), dt.float32r, kind="ExternalInput"
    )
    wt_d = nc.dram_tensor("wt", (128, NBLK * 128), dt.float32r, kind="ExternalInput")
    b_d = nc.dram_tensor("bias", (128, NM), dt.float32, kind="ExternalInput")
    out_d = nc.dram_tensor(
        "out", (IMG_PER_CORE, O, 2 * H, 2 * W), dt.float32, kind="ExternalOutput"
    )

    with tile.TileContext(nc) as tc:
        with (
            tc.tile_pool(name="xpool", bufs=1) as xpool,
            tc.tile_pool(name="wpool", bufs=1) as wpool,
            tc.tile_pool(name="zpool", bufs=3) as zpool,
            tc.tile_pool(name="spool", bufs=4) as spool,
            tc.tile_pool(name="opool", bufs=4) as opool,
            tc.tile_pool(name="psum", bufs=8, space="PSUM") as psum_pool,
        ):
            wt_t = wpool.tile([128, NBLK * 128], dt.float32r, tag="wt")
            b_t = wpool.tile([128, NM], dt.float32, tag="bias")
            zero_t = wpool.tile([128, 2 * UROWS], dt.float32, tag="zero")
            xp_t = {}

            def load_xp(img, k, split=False):
                t = xpool.tile([128, HPR, WPC], dt.float32r, tag=f"xp{img}{k}")
                src = xp_d.ap()[img, k].rearrange("p (h w) -> p h w", h=HPR)
                if split:
                    nc.sync.dma_start(t[:, :12, :], src[:, :12, :])
                    nc.sync.dma_start(t[:, 12:, :], src[:, 12:, :])
                else:
                    nc.sync.dma_start(t[:], src)
                xp_t[img, k] = t

            # minimal working set first: first tile touches xp(0,*) top rows
            # and wt m0 blocks only.
            load_xp(0, 0, split=True)
            nc.sync.dma_start(wt_t[:, : 36 * 128], wt_d.ap()[:, : 36 * 128])
            nc.sync.dma_start(b_t[:], b_d.ap()[:])
            nc.vector.memset(zero_t[:], 0.0)
            load_xp(0, 1)
            nc.sync.dma_start(wt_t[:, 36 * 128 :], wt_d.ap()[:, 36 * 128 :])
            load_xp(1, 0)
            load_xp(1, 1)

            zero_v = zero_t[:].rearrange("p (u a) -> p u a", a=2)
            tile_idx = 0
            for img in range(IMG_PER_CORE):
                for m in range(NM):
                    for ub in range(UB):
                        U0 = ub * UROWS
                        z_sb = zpool.tile([128, 2 * UROWS, ZW], dt.float32)
                        z_v = z_sb[:].rearrange(
                            "p (u a) (v b) -> p u a v b", a=2, b=2
                        )
                        # structural-zero pad cols cs=0 / cs=130 get delta
                        nc.scalar.activation(
                            z_v[:, :, :, 0, 0],
                            zero_v,
                            mybir.ActivationFunctionType.Identity,
                            bias=b_t[:, m : m + 1],
                        )
                        nc.scalar.activation(
                            z_v[:, :, :, 65, 0],
                            zero_v,
                            mybir.ActivationFunctionType.Identity,
                            bias=b_t[:, m : m + 1],
                        )
                        for A in range(2):
                            for B in range(2):
                                ncol = NCOL[B]
                                acc = psum_pool.tile([128, UROWS, ncol], dt.float32)
                                ntot = 2 * 3 * (2 - B)
                                n = 0
                                for k in range(NK):
                                    for ti in range(3):
                                        for tj in range(2 - B):
                                            du, dv = DU[ti], DV[B][tj]
                                            idx = _blk(m, A, B, k, ti, tj)
                                            rhs = xp_t[img, k][
                                                :,
                                                U0 + du + 1 : U0 + du + 1 + UROWS,
                                                dv + 1 : dv + 1 + ncol,
                                            ]
                                            nc.tensor.matmul(
                                                acc[:],
                                                wt_t[
                                                    :, idx * 128 : (idx + 1) * 128
                                                ],
                                                rhs,
                                                start=(n == 0),
                                                stop=(n == ntot - 1),
                                            )
                                            n += 1
                                # B=0 -> odd cs (b=1, v=V 0..64); B=1 -> even
                                # cs (b=0, v=V+1 1..64)
                                if B == 0:
                                    dst = z_v[:, :, A, 0:66, 1]
                                else:
                                    dst = z_v[:, :, A, 1:65, 0]
                                nc.scalar.activation(
                                    dst,
                                    acc[:],
                                    mybir.ActivationFunctionType.Identity,
                                    bias=b_t[:, m : m + 1],
                                )
                        # column FIR: out = 1*z[C] + 3*z[C+1] + 3*z[C+2] + 1*z[C+3]
                        # Pool does the two adds (tensor_add is Pool-legal),
                        # DVE the fused 3*s2+s1 (TensorScalarPtr is DVE-only).
                        s1 = spool.tile([128, 2 * UROWS, 2 * W], dt.float32)
                        s2 = spool.tile([128, 2 * UROWS, 2 * W], dt.float32)
                        out_sb = opool.tile([128, 2 * UROWS, 2 * W], dt.float32)
                        nc.gpsimd.tensor_add(
                            s1[:], z_sb[:, :, 0:128], z_sb[:, :, 3:131]
                        )
                        nc.gpsimd.tensor_add(
                            s2[:], z_sb[:, :, 1:129], z_sb[:, :, 2:130]
                        )
                        nc.vector.scalar_tensor_tensor(
                            out_sb[:],
                            s2[:],
                            3.0,
                            s1[:],
                            mybir.AluOpType.mult,
                            mybir.AluOpType.add,
                        )
                        nc.sync.dma_start(
                            out_d.ap()[
                                img,
                                m * 128 : (m + 1) * 128,
                                8 * ub : 8 * ub + 8,
                                :,
                            ],
                            out_sb[:],
                        )
                        tile_idx += 1

    nc.compile()
    return nc


def _compose_weights(w):
    """w (256,256,3,3) -> [128 ch_local, 72*128] float32, row-FIR folded and
    the col-FIR 1/4 gain pre-applied."""
    g = np.array([1.0, 3.0, 3.0, 1.0], dtype=np.float64) / 4.0
    w64 = w.astype(np.float64)
    Wr = np.zeros((O, C, 6, 3), dtype=np.float64)
    for s in range(4):
        Wr[:, :, s : s + 3, :] += g[s] * w64
    Wpe = 0.25 * Wr
    blocks = np.empty((NBLK, 128, 128), dtype=np.float32)  # [blk, ch_l, o_l]
    for m in range(NM):
        for A in range(2):
            for B in range(2):
                for k in range(NK):
                    for ti in range(3):
                        for tj in range(2 - B):
                            sub = Wpe[
                                m * 128 : (m + 1) * 128,
                                k * 128 : (k + 1) * 128,
                                PL[A][ti],
                                JL[B][tj],
                            ]
                            blocks[_blk(m, A, B, k, ti, tj)] = sub.T.astype(
                                np.float32
                            )
    return np.ascontiguousarray(blocks.transpose(1, 0, 2)).reshape(128, -1)


def kernel(x, w, b):
    global _compiled, LAST_RESULTS
    if _compiled is None:
        _compiled = _build()
    nc = _compiled

    x = np.asarray(x, dtype=np.float32)
    w = np.asarray(w, dtype=np.float32)
    b = np.asarray(b, dtype=np.float32)

    wt = _compose_weights(w)
    b2 = np.ascontiguousarray(b.reshape(NM, 128).T.astype(np.float32)) / 8.0
    xp = np.pad(x, ((0, 0), (0, 0), (1, 1), (1, 2)))  # (16, 256, 66, 67)
    xp = np.ascontiguousarray(xp.reshape(N_CORES, IMG_PER_CORE, NK, 128, HPR * WPC))

    in_maps = [{"xp": xp[core], "wt": wt, "bias": b2} for core in range(N_CORES)]
    try:
        res = run_bass_kernel_spmd(nc, in_maps, list(range(N_CORES)))
    except ModuleNotFoundError:
        # BASS_TRACE set in an env without the axon NTFF hook module —
        # retry with tracing disabled.
        import os

        os.environ["BASS_NEVER_TRACE"] = "1"
        res = run_bass_kernel_spmd(nc, in_maps, list(range(N_CORES)))
    LAST_RESULTS = res
    out = np.concatenate([res.results[i]["out"] for i in range(N_CORES)], axis=0)
    return out


# revision 5
# speedup vs baseline: 1.8354x; 1.1481x over previous
"""StyleGAN2 fused upsample2x + 3x3 conv + FIR(1,3,3,1) + bias — TRN2 Bass kernel.

v3 — split the separable FIR: the ROW pass is folded into the conv weights,
the COLUMN pass runs on the vector engines.  Math:

  y1 = corr(zero-dilate-2(x), w, full pad)            # (129,129) per channel
  z[r,c'] = sum_s g[s] y1[r-1+s, c']   g=(1,3,3,1)/4  # row FIR -> 128 rows
  out[R,C] = sum_t g[t] z[R, C-1+t] + bias            # col FIR -> 128 cols

Folding the row FIR into w gives Wr (6 rows x 3 cols).  With output row
parity A = r%2 only 3 row taps survive (p in (1,3,5) for A=0, (0,2,4) for
A=1, all at u-offsets (-1,0,1)); with column parity B = c'%2 the zero
dilation leaves 2 col taps (B=0) or 1 (B=1): 18 matmul taps per output
pixel instead of 36 — half the PE work of composing the full 6x6 kernel.

The 4-tap column FIR is symmetric, so with the 1/4 gain folded into the
weights it is 3 fused element-wise ops: s1 = z[C]+z[C+3], s2 = z[C+1]+z[C+2],
out = 3*s2 + s1.  Pool (GpSimd) computes s1 (tensor_tensor is ~2.2cyc/elem
there), DVE computes s2 and the fused multiply-add (TensorScalarPtr is
DVE-only).  Bias is folded as delta = bias/8 added to every stored z column
(including the two structural-zero pad columns): integer col taps sum to 8.

Tiling: o=256 in 2x128 (M), contraction 256 in 2x128 (K), spatial as 8
macro blocks of 8 U-rows (16 output rows).  Per macro and output parity A:
B=0 runs as two 4-row PSUM groups of [128,4,66] (264 free; one PSUM bank
is 512 fp32) and B=1 as one 8-row group [128,8,64] (512 free) — frees
>=256 keep float32r at 1 cycle/row, and fp32r requires even inner dims.
Data-parallel over batch: 2 images/core, 8 cores.
"""

import sys

sys.path.insert(0, "/opt/trn_rl_repo")

import numpy as np

import concourse.bacc as bacc
import concourse.mybir as mybir
import concourse.tile as tile
from concourse.bass_utils import run_bass_kernel_spmd

N_CORES = 8
IMGS = 16
IMG_PER_CORE = IMGS // N_CORES  # 2
C = 256  # in channels
O = 256  # out channels
H = W = 64
NK = C // 128  # 2 contraction splits
NM = O // 128  # 2 output-partition splits
HPR = H + 2  # 66: padded input rows (1 halo each side)
WPC = H + 3  # 67: padded input cols (1 left, 2 right: fp32r needs even dims)
MB = 8  # macro row blocks
MROWS = 8  # U rows per macro block (16 output rows)
ZW = 132  # stored z width: cs=0..130 used (cs=c'+1, pads at 0 and 130)
NBLK = NM * 18 * NK  # 72 weight blocks of 128x128

PL = [(1, 3, 5), (0, 2, 4)]  # Wr row index by (A, ti)
JL = [(0, 2), (1,)]  # Wr col index by (B, tj)
DU = (-1, 0, 1)  # x row offset by ti (same for both A)
DV = [(-1, 0), (0,)]  # x col offset by (B, tj)
NCOL = (66, 64)  # z cols per (B); B=0 col 65 is pad-driven slack (cs=131)


def _blk(m, A, B, k, ti, tj):
    base = m * 36 + A * 18 + (12 if B else 0)
    return base + (k * 3 + ti) * (2 - B) + tj


_compiled = None
LAST_RESULTS = None


def _build():
    nc = bacc.Bacc(None, target_bir_lowering=False, debug=False)
    dt = mybir.dt

    xp_d = nc.dram_tensor(
        "xp", (IMG_PER_CORE, NK, 128, HPR * WPC), dt.float32r, kind="ExternalInput"
    )
    wt_d = nc.dram_tensor("wt", (128, NBLK * 128), dt.float32r, kind="ExternalInput")
    b_d = nc.dram_tensor("bias", (128, NM), dt.float32, kind="ExternalInput")
    out_d = nc.dram_tensor(
        "out", (IMG_PER_CORE, O, 2 * H, 2 * W), dt.float32, kind="ExternalOutput"
    )

    with tile.TileContext(nc) as tc:
        with (
            tc.tile_pool(name="xpool", bufs=1) as xpool,
            tc.tile_pool(name="wpool", bufs=1) as wpool,
            tc.tile_pool(name="zpool", bufs=3) as zpool,
            tc.tile_pool(name="spool", bufs=2) as spool,
            tc.tile_pool(name="opool", bufs=3) as opool,
            tc.tile_pool(name="psum", bufs=8, space="PSUM") as psum_pool,
        ):
            wt_t = wpool.tile([128, NBLK * 128], dt.float32r, tag="wt")
            b_t = wpool.tile([128, NM], dt.float32, tag="bias")
            zero_t = wpool.tile([128, 2 * MROWS], dt.float32, tag="zero")
            xp_t = {}

            def load_xp(img, k, split=False):
                t = xpool.tile([128, HPR, WPC], dt.float32r, tag=f"xp{img}{k}")
                src = xp_d.ap()[img, k].rearrange("p (h w) -> p h w", h=HPR)
                if split:
                    nc.sync.dma_start(t[:, :12, :], src[:, :12, :])
                    nc.sync.dma_start(t[:, 12:, :], src[:, 12:, :])
                else:
                    nc.sync.dma_start(t[:], src)
                xp_t[img, k] = t

            def load_wt(lo, hi):
                nc.sync.dma_start(
                    wt_t[:, lo * 128 : hi * 128], wt_d.ap()[:, lo * 128 : hi * 128]
                )

            # minimal working set first: the first PSUM group touches only
            # blocks 0..11 (m0/A0/B0) and xp(0,*) top rows.
            load_xp(0, 0, split=True)
            load_wt(0, 12)
            nc.sync.dma_start(b_t[:], b_d.ap()[:])
            nc.vector.memset(zero_t[:], 0.0)
            load_xp(0, 1)
            load_wt(12, 18)
            load_wt(18, 30)
            load_wt(30, 36)
            load_wt(36, 72)
            load_xp(1, 0)
            load_xp(1, 1)

            zero_v = zero_t[:].rearrange("p (u a) -> p u a", a=2)
            for img in range(IMG_PER_CORE):
                for m in range(NM):
                    for mb in range(MB):
                        U0 = mb * MROWS
                        z_sb = zpool.tile([128, 2 * MROWS, ZW], dt.float32)
                        z_v = z_sb[:].rearrange(
                            "p (u a) (v b) -> p u a v b", a=2, b=2
                        )
                        # structural-zero pad cols cs=0 / cs=130 get delta
                        nc.scalar.activation(
                            z_v[:, :, :, 0, 0],
                            zero_v,
                            mybir.ActivationFunctionType.Identity,
                            bias=b_t[:, m : m + 1],
                        )
                        nc.scalar.activation(
                            z_v[:, :, :, 65, 0],
                            zero_v,
                            mybir.ActivationFunctionType.Identity,
                            bias=b_t[:, m : m + 1],
                        )

                        def mm_group(A, B, u0, rows, dst):
                            ncol = NCOL[B]
                            acc = psum_pool.tile([128, rows, ncol], dt.float32)
                            ntot = 2 * 3 * (2 - B)
                            n = 0
                            for k in range(NK):
                                for ti in range(3):
                                    for tj in range(2 - B):
                                        du, dv = DU[ti], DV[B][tj]
                                        idx = _blk(m, A, B, k, ti, tj)
                                        rhs = xp_t[img, k][
                                            :,
                                            u0 + du + 1 : u0 + du + 1 + rows,
                                            dv + 1 : dv + 1 + ncol,
                                        ]
                                        nc.tensor.matmul(
                                            acc[:],
                                            wt_t[:, idx * 128 : (idx + 1) * 128],
                                            rhs,
                                            start=(n == 0),
                                            stop=(n == ntot - 1),
                                        )
                                        n += 1
                            nc.scalar.activation(
                                dst,
                                acc[:],
                                mybir.ActivationFunctionType.Identity,
                                bias=b_t[:, m : m + 1],
                            )

                        for A in range(2):
                            # B=0: two 4-row groups (264 free each, bank cap);
                            # B=1: one 8-row group (512 free).
                            mm_group(A, 0, U0, 4, z_v[:, 0:4, A, 0:66, 1])
                            mm_group(A, 0, U0 + 4, 4, z_v[:, 4:8, A, 0:66, 1])
                            mm_group(A, 1, U0, 8, z_v[:, 0:8, A, 1:65, 0])

                        # column FIR: out = 1*z[C] + 3*z[C+1] + 3*z[C+2] + 1*z[C+3]
                        s1 = spool.tile([128, 2 * MROWS, 2 * W], dt.float32)
                        s2 = spool.tile([128, 2 * MROWS, 2 * W], dt.float32)
                        out_sb = opool.tile([128, 2 * MROWS, 2 * W], dt.float32)
                        nc.gpsimd.tensor_add(
                            s1[:], z_sb[:, :, 0:128], z_sb[:, :, 3:131]
                        )
                        nc.vector.tensor_add(
                            s2[:], z_sb[:, :, 1:129], z_sb[:, :, 2:130]
                        )
                        nc.vector.scalar_tensor_tensor(
                            out_sb[:],
                            s2[:],
                            3.0,
                            s1[:],
                            mybir.AluOpType.mult,
                            mybir.AluOpType.add,
                        )
                        nc.sync.dma_start(
                            out_d.ap()[
                                img,
                                m * 128 : (m + 1) * 128,
                                16 * mb : 16 * mb + 16,
                                :,
                            ],
                            out_sb[:],
                        )

    nc.compile()
    return nc


def _compose_weights(w):
    """w (256,256,3,3) -> [128 ch_local, 72*128] float32, row-FIR folded and
    the col-FIR 1/4 gain pre-applied."""
    g = np.array([1.0, 3.0, 3.0, 1.0], dtype=np.float64) / 4.0
    w64 = w.astype(np.float64)
    Wr = np.zeros((O, C, 6, 3), dtype=np.float64)
    for s in range(4):
        Wr[:, :, s : s + 3, :] += g[s] * w64
    Wpe = 0.25 * Wr
    blocks = np.empty((NBLK, 128, 128), dtype=np.float32)  # [blk, ch_l, o_l]
    for m in range(NM):
        for A in range(2):
            for B in range(2):
                for k in range(NK):
                    for ti in range(3):
                        for tj in range(2 - B):
                            sub = Wpe[
                                m * 128 : (m + 1) * 128,
                                k * 128 : (k + 1) * 128,
                                PL[A][ti],
                                JL[B][tj],
                            ]
                            blocks[_blk(m, A, B, k, ti, tj)] = sub.T.astype(
                                np.float32
                            )
    return np.ascontiguousarray(blocks.transpose(1, 0, 2)).reshape(128, -1)


def kernel(x, w, b):
    global _compiled, LAST_RESULTS
    if _compiled is None:
        _compiled = _build()
    nc = _compiled

    x = np.asarray(x, dtype=np.float32)
    w = np.asarray(w, dtype=np.float32)
    b = np.asarray(b, dtype=np.float32)

    wt = _compose_weights(w)
    b2 = np.ascontiguousarray(b.reshape(NM, 128).T.astype(np.float32)) / 8.0
    xp = np.pad(x, ((0, 0), (0, 0), (1, 1), (1, 2)))  # (16, 256, 66, 67)
    xp = np.ascontiguousarray(xp.reshape(N_CORES, IMG_PER_CORE, NK, 128, HPR * WPC))

    in_maps = [{"xp": xp[core], "wt": wt, "bias": b2} for core in range(N_CORES)]
    try:
        res = run_bass_kernel_spmd(nc, in_maps, list(range(N_CORES)))
    except ModuleNotFoundError:
        # BASS_TRACE set in an env without the axon NTFF hook module —
        # retry with tracing disabled.
        import os

        os.environ["BASS_NEVER_TRACE"] = "1"
        res = run_bass_kernel_spmd(nc, in_maps, list(range(N_CORES)))
    LAST_RESULTS = res
    out = np.concatenate([res.results[i]["out"] for i in range(N_CORES)], axis=0)
    return out


# revision 7
# speedup vs baseline: 1.8453x; 1.0054x over previous
"""StyleGAN2 fused upsample2x + 3x3 conv + FIR(1,3,3,1) + bias — TRN2 Bass kernel.

v4 — split the separable FIR: the ROW pass is folded into the conv weights,
the COLUMN pass runs on the vector engines.  Math:

  y1 = corr(zero-dilate-2(x), w, full pad)            # (129,129) per channel
  z[r,c'] = sum_s g[s] y1[r-1+s, c']   g=(1,3,3,1)/4  # row FIR -> 128 rows
  out[R,C] = sum_t g[t] z[R, C-1+t] + bias            # col FIR -> 128 cols

Folding the row FIR into w gives Wr (6 rows x 3 cols).  With output row
parity A = r%2 only 3 row taps survive (p in (1,3,5) for A=0, (0,2,4) for
A=1, all at u-offsets (-1,0,1)); with column parity B = c'%2 the zero
dilation leaves 2 col taps (B=0) or 1 (B=1): 18 matmul taps per output
pixel instead of 36 — half the PE work of composing the full 6x6 kernel.

Matmuls run in bf16 (inputs rounded host-side): same 1 cycle/row as
float32r but the self-loading LDWEIGHTS phase drops to 1 cycle/row so it
fully hides behind the 264-row moving pass, and input DMA bytes halve.
PSUM accumulation stays fp32; error lands ~5e-3, well inside the 2e-2 gate.

The 4-tap column FIR is symmetric: with the 1/4 gain folded into the
weights it is 3 fused element-wise ops: s1 = z[C]+z[C+3], s2 = z[C+1]+z[C+2],
out = 3*s2 + s1.  Pool (GpSimd) computes s1 (~2.2cyc/elem for
tensor_tensor), DVE computes s2 and the fused multiply-add (TensorScalarPtr
is DVE-only).  Bias is folded as delta = bias/8 on every stored z column
(including the two structural-zero pad columns): integer col taps sum to 8.

Tiling: o=256 in 2x128 (M), contraction 256 in 2x128 (K), spatial as 8
macro blocks of 8 U-rows (16 output rows).  Per macro and parity A: B=0 is
two 4-row PSUM groups [128,4,66] (264 free; PSUM bank = 512 fp32), B=1 one
8-row group [128,8,64] (512 free).  Groups are ordered so the first 8
output rows complete early and the column FIR + store run in two 8-row
halves overlapped with the PE.  Data-parallel: 2 images/core, 8 cores.
"""

import sys

sys.path.insert(0, "/opt/trn_rl_repo")

import ml_dtypes
import numpy as np

import concourse.bacc as bacc
import concourse.mybir as mybir
import concourse.tile as tile
from concourse.bass_utils import run_bass_kernel_spmd

N_CORES = 8
IMGS = 16
IMG_PER_CORE = IMGS // N_CORES  # 2
C = 256  # in channels
O = 256  # out channels
H = W = 64
NK = C // 128  # 2 contraction splits
NM = O // 128  # 2 output-partition splits
HPR = H + 2  # 66: padded input rows (1 halo each side)
WPC = H + 3  # 67: padded input cols (1 left, 2 right)
MB = 8  # macro row blocks
MROWS = 8  # U rows per macro block (16 output rows)
ZW = 132  # stored z width: cs=0..130 used (cs=c'+1, pads at 0 and 130)
NBLK = NM * 18 * NK  # 72 weight blocks of 128x128

PL = [(1, 3, 5), (0, 2, 4)]  # Wr row index by (A, ti)
JL = [(0, 2), (1,)]  # Wr col index by (B, tj)
DU = (-1, 0, 1)  # x row offset by ti (same for both A)
DV = [(-1, 0), (0,)]  # x col offset by (B, tj)
NCOL = (66, 64)  # z cols per (B); B=0 col 65 is pad-driven slack (cs=131)


def _blk(m, A, B, k, ti, tj):
    base = m * 36 + A * 18 + (12 if B else 0)
    return base + (k * 3 + ti) * (2 - B) + tj


_compiled = None
LAST_RESULTS = None


def _build():
    nc = bacc.Bacc(None, target_bir_lowering=False, debug=False)
    dt = mybir.dt

    xp_d = nc.dram_tensor(
        "xp", (IMG_PER_CORE, NK, 128, HPR * WPC), dt.bfloat16, kind="ExternalInput"
    )
    wt_d = nc.dram_tensor("wt", (128, NBLK * 128), dt.bfloat16, kind="ExternalInput")
    b_d = nc.dram_tensor("bias", (128, NM), dt.float32, kind="ExternalInput")
    out_d = nc.dram_tensor(
        "out", (IMG_PER_CORE, O, 2 * H, 2 * W), dt.float32, kind="ExternalOutput"
    )

    with tile.TileContext(nc) as tc:
        with (
            tc.tile_pool(name="xpool", bufs=1) as xpool,
            tc.tile_pool(name="wpool", bufs=1) as wpool,
            tc.tile_pool(name="zpool", bufs=4) as zpool,
            tc.tile_pool(name="spool", bufs=4) as spool,
            tc.tile_pool(name="opool", bufs=6) as opool,
            tc.tile_pool(name="psum", bufs=8, space="PSUM") as psum_pool,
        ):
            wt_t = wpool.tile([128, NBLK * 128], dt.bfloat16, tag="wt")
            b_t = wpool.tile([128, NM], dt.float32, tag="bias")
            zero_t = wpool.tile([128, 2 * MROWS], dt.float32, tag="zero")
            xp_t = {}

            def load_xp_rows(img, k, r0, r1):
                if (img, k) not in xp_t:
                    t = xpool.tile([128, HPR, WPC], dt.bfloat16, tag=f"xp{img}{k}")
                    xp_t[img, k] = t
                t = xp_t[img, k]
                src = xp_d.ap()[img, k].rearrange("p (h w) -> p h w", h=HPR)
                nc.sync.dma_start(t[:, r0:r1, :], src[:, r0:r1, :])

            def load_wt(lo, hi):
                nc.sync.dma_start(
                    wt_t[:, lo * 128 : hi * 128], wt_d.ap()[:, lo * 128 : hi * 128]
                )

            # minimal working set first: group order per macro is
            # A0B0lo(blk 0-11), A1B0lo(18-29), A0B1(12-17), A1B1(30-35),
            # A0B0hi, A1B0hi; first groups touch xp rows 0:6, B1 rows 0:10.
            load_xp_rows(0, 0, 0, 6)
            load_wt(0, 12)
            load_xp_rows(0, 1, 0, 6)
            nc.sync.dma_start(b_t[:], b_d.ap()[:])
            nc.vector.memset(zero_t[:], 0.0)
            load_wt(18, 30)
            load_xp_rows(0, 0, 6, 18)
            load_xp_rows(0, 1, 6, 18)
            load_wt(12, 18)
            load_wt(30, 36)
            load_xp_rows(0, 0, 18, HPR)
            load_xp_rows(0, 1, 18, HPR)
            load_wt(36, 54)
            load_wt(54, 72)
            load_xp_rows(1, 0, 0, HPR)
            load_xp_rows(1, 1, 0, HPR)

            zero_v = zero_t[:].rearrange("p (u a) -> p u a", a=2)
            for img in range(IMG_PER_CORE):
                for m in range(NM):
                    for mb in range(MB):
                        U0 = mb * MROWS
                        z_sb = zpool.tile([128, 2 * MROWS, ZW], dt.float32)
                        z_v = z_sb[:].rearrange(
                            "p (u a) (v b) -> p u a v b", a=2, b=2
                        )
                        # structural-zero pad cols cs=0 / cs=130 get delta
                        nc.scalar.activation(
                            z_v[:, :, :, 0, 0],
                            zero_v,
                            mybir.ActivationFunctionType.Identity,
                            bias=b_t[:, m : m + 1],
                        )
                        nc.scalar.activation(
                            z_v[:, :, :, 65, 0],
                            zero_v,
                            mybir.ActivationFunctionType.Identity,
                            bias=b_t[:, m : m + 1],
                        )

                        def mm_group(A, B, u0, rows, dst):
                            ncol = NCOL[B]
                            acc = psum_pool.tile([128, rows, ncol], dt.float32)
                            ntot = 2 * 3 * (2 - B)
                            n = 0
                            for k in range(NK):
                                for ti in range(3):
                                    for tj in range(2 - B):
                                        du, dv = DU[ti], DV[B][tj]
                                        idx = _blk(m, A, B, k, ti, tj)
                                        rhs = xp_t[img, k][
                                            :,
                                            u0 + du + 1 : u0 + du + 1 + rows,
                                            dv + 1 : dv + 1 + ncol,
                                        ]
                                        nc.tensor.matmul(
                                            acc[:],
                                            wt_t[:, idx * 128 : (idx + 1) * 128],
                                            rhs,
                                            start=(n == 0),
                                            stop=(n == ntot - 1),
                                        )
                                        n += 1
                            nc.scalar.activation(
                                dst,
                                acc[:],
                                mybir.ActivationFunctionType.Identity,
                                bias=b_t[:, m : m + 1],
                            )

                        def col_fir(half):
                            r0 = half * MROWS  # z/out row offset within macro
                            s1 = spool.tile([128, MROWS, 2 * W], dt.float32)
                            s2 = spool.tile([128, MROWS, 2 * W], dt.float32)
                            o_sb = opool.tile([128, MROWS, 2 * W], dt.float32)
                            zs = z_sb[:, r0 : r0 + MROWS, :]
                            nc.gpsimd.tensor_add(
                                s1[:], zs[:, :, 0:128], zs[:, :, 3:131]
                            )
                            nc.vector.tensor_add(
                                s2[:], zs[:, :, 1:129], zs[:, :, 2:130]
                            )
                            nc.vector.scalar_tensor_tensor(
                                o_sb[:],
                                s2[:],
                                3.0,
                                s1[:],
                                mybir.AluOpType.mult,
                                mybir.AluOpType.add,
                            )
                            nc.sync.dma_start(
                                out_d.ap()[
                                    img,
                                    m * 128 : (m + 1) * 128,
                                    16 * mb + r0 : 16 * mb + r0 + MROWS,
                                    :,
                                ],
                                o_sb[:],
                            )

                        # first half's rows (u 0:4, both A) complete after the
                        # two B0lo groups + both full-height B1 groups.
                        mm_group(0, 0, U0, 4, z_v[:, 0:4, 0, 0:66, 1])
                        mm_group(1, 0, U0, 4, z_v[:, 0:4, 1, 0:66, 1])
                        mm_group(0, 1, U0, 8, z_v[:, 0:8, 0, 1:65, 0])
                        mm_group(1, 1, U0, 8, z_v[:, 0:8, 1, 1:65, 0])
                        col_fir(0)
                        mm_group(0, 0, U0 + 4, 4, z_v[:, 4:8, 0, 0:66, 1])
                        mm_group(1, 0, U0 + 4, 4, z_v[:, 4:8, 1, 0:66, 1])
                        col_fir(1)

    nc.compile()
    return nc


def _compose_weights(w):
    """w (256,256,3,3) -> [128 ch_local, 72*128] bf16, row-FIR folded and
    the col-FIR 1/4 gain pre-applied."""
    g = np.array([1.0, 3.0, 3.0, 1.0], dtype=np.float64) / 4.0
    w64 = w.astype(np.float64)
    Wr = np.zeros((O, C, 6, 3), dtype=np.float64)
    for s in range(4):
        Wr[:, :, s : s + 3, :] += g[s] * w64
    Wpe = 0.25 * Wr
    blocks = np.empty((NBLK, 128, 128), dtype=np.float32)  # [blk, ch_l, o_l]
    for m in range(NM):
        for A in range(2):
            for B in range(2):
                for k in range(NK):
                    for ti in range(3):
                        for tj in range(2 - B):
                            sub = Wpe[
                                m * 128 : (m + 1) * 128,
                                k * 128 : (k + 1) * 128,
                                PL[A][ti],
                                JL[B][tj],
                            ]
                            blocks[_blk(m, A, B, k, ti, tj)] = sub.T.astype(
                                np.float32
                            )
    return np.ascontiguousarray(blocks.transpose(1, 0, 2)).reshape(128, -1)


def kernel(x, w, b):
    global _compiled, LAST_RESULTS
    if _compiled is None:
        _compiled = _build()
    nc = _compiled

    x = np.asarray(x, dtype=np.float32)
    w = np.asarray(w, dtype=np.float32)
    b = np.asarray(b, dtype=np.float32)

    wt = _compose_weights(w).astype(ml_dtypes.bfloat16)
    b2 = np.ascontiguousarray(b.reshape(NM, 128).T.astype(np.float32)) / 8.0
    xp = np.pad(x, ((0, 0), (0, 0), (1, 1), (1, 2)))  # (16, 256, 66, 67)
    xp = np.ascontiguousarray(
        xp.reshape(N_CORES, IMG_PER_CORE, NK, 128, HPR * WPC)
    ).astype(ml_dtypes.bfloat16)

    in_maps = [{"xp": xp[core], "wt": wt, "bias": b2} for core in range(N_CORES)]
    try:
        res = run_bass_kernel_spmd(nc, in_maps, list(range(N_CORES)))
    except ModuleNotFoundError:
        # BASS_TRACE set in an env without the axon NTFF hook module —
        # retry with tracing disabled.
        import os

        os.environ["BASS_NEVER_TRACE"] = "1"
        res = run_bass_kernel_spmd(nc, in_maps, list(range(N_CORES)))
    LAST_RESULTS = res
    out = np.concatenate([res.results[i]["out"] for i in range(N_CORES)], axis=0)
    return out


# revision 8
# speedup vs baseline: 1.8463x; 1.0005x over previous
"""StyleGAN2 fused upsample2x + 3x3 conv + FIR(1,3,3,1) + bias — TRN2 Bass kernel.

v4 — split the separable FIR: the ROW pass is folded into the conv weights,
the COLUMN pass runs on the vector engines.  Math:

  y1 = corr(zero-dilate-2(x), w, full pad)            # (129,129) per channel
  z[r,c'] = sum_s g[s] y1[r-1+s, c']   g=(1,3,3,1)/4  # row FIR -> 128 rows
  out[R,C] = sum_t g[t] z[R, C-1+t] + bias            # col FIR -> 128 cols

Folding the row FIR into w gives Wr (6 rows x 3 cols).  With output row
parity A = r%2 only 3 row taps survive (p in (1,3,5) for A=0, (0,2,4) for
A=1, all at u-offsets (-1,0,1)); with column parity B = c'%2 the zero
dilation leaves 2 col taps (B=0) or 1 (B=1): 18 matmul taps per output
pixel instead of 36 — half the PE work of composing the full 6x6 kernel.

Matmuls run in bf16 (inputs rounded host-side): same 1 cycle/row as
float32r but the self-loading LDWEIGHTS phase drops to 1 cycle/row so it
fully hides behind the 264-row moving pass, and input DMA bytes halve.
PSUM accumulation stays fp32; error lands ~5e-3, well inside the 2e-2 gate.

The 4-tap column FIR is symmetric: with the 1/4 gain folded into the
weights it is 3 fused element-wise ops: s1 = z[C]+z[C+3], s2 = z[C+1]+z[C+2],
out = 3*s2 + s1.  Pool (GpSimd) computes s1 (~2.2cyc/elem for
tensor_tensor), DVE computes s2 and the fused multiply-add (TensorScalarPtr
is DVE-only).  Bias is folded as delta = bias/8 on every stored z column
(including the two structural-zero pad columns): integer col taps sum to 8.

Tiling: o=256 in 2x128 (M), contraction 256 in 2x128 (K), spatial as 8
macro blocks of 8 U-rows (16 output rows).  Per macro and parity A: B=0 is
two 4-row PSUM groups [128,4,66] (264 free; PSUM bank = 512 fp32), B=1 one
8-row group [128,8,64] (512 free).  Groups are ordered so the first 8
output rows complete early and the column FIR + store run in two 8-row
halves overlapped with the PE.  Data-parallel: 2 images/core, 8 cores.
"""

import sys

sys.path.insert(0, "/opt/trn_rl_repo")

import ml_dtypes
import numpy as np

import concourse.bacc as bacc
import concourse.mybir as mybir
import concourse.tile as tile
from concourse.bass_utils import run_bass_kernel_spmd

N_CORES = 8
IMGS = 16
IMG_PER_CORE = IMGS // N_CORES  # 2
C = 256  # in channels
O = 256  # out channels
H = W = 64
NK = C // 128  # 2 contraction splits
NM = O // 128  # 2 output-partition splits
HPR = H + 2  # 66: padded input rows (1 halo each side)
WPC = H + 3  # 67: padded input cols (1 left, 2 right)
MB = 8  # macro row blocks
MROWS = 8  # U rows per macro block (16 output rows)
ZW = 132  # stored z width: cs=0..130 used (cs=c'+1, pads at 0 and 130)
NBLK = NM * 18 * NK  # 72 weight blocks of 128x128

PL = [(1, 3, 5), (0, 2, 4)]  # Wr row index by (A, ti)
JL = [(0, 2), (1,)]  # Wr col index by (B, tj)
DU = (-1, 0, 1)  # x row offset by ti (same for both A)
DV = [(-1, 0), (0,)]  # x col offset by (B, tj)
NCOL = (66, 64)  # z cols per (B); B=0 col 65 is pad-driven slack (cs=131)


def _blk(m, A, B, k, ti, tj):
    base = m * 36 + A * 18 + (12 if B else 0)
    return base + (k * 3 + ti) * (2 - B) + tj


_compiled = None
LAST_RESULTS = None


def _build():
    nc = bacc.Bacc(None, target_bir_lowering=False, debug=False)
    dt = mybir.dt

    xp_d = nc.dram_tensor(
        "xp", (IMG_PER_CORE, NK, 128, HPR * WPC), dt.bfloat16, kind="ExternalInput"
    )
    wt_d = nc.dram_tensor("wt", (128, NBLK * 128), dt.bfloat16, kind="ExternalInput")
    b_d = nc.dram_tensor("bias", (128, NM), dt.float32, kind="ExternalInput")
    out_d = nc.dram_tensor(
        "out", (IMG_PER_CORE, O, 2 * H, 2 * W), dt.float32, kind="ExternalOutput"
    )

    with tile.TileContext(nc) as tc:
        with (
            tc.tile_pool(name="xpool", bufs=1) as xpool,
            tc.tile_pool(name="wpool", bufs=1) as wpool,
            tc.tile_pool(name="zpool", bufs=6) as zpool,
            tc.tile_pool(name="spool", bufs=6) as spool,
            tc.tile_pool(name="opool", bufs=8) as opool,
            tc.tile_pool(name="psum", bufs=8, space="PSUM") as psum_pool,
        ):
            wt_t = wpool.tile([128, NBLK * 128], dt.bfloat16, tag="wt")
            b_t = wpool.tile([128, NM], dt.float32, tag="bias")
            zero_t = wpool.tile([128, 2 * MROWS], dt.float32, tag="zero")
            xp_t = {}

            def load_xp_rows(img, k, r0, r1):
                if (img, k) not in xp_t:
                    t = xpool.tile([128, HPR, WPC], dt.bfloat16, tag=f"xp{img}{k}")
                    xp_t[img, k] = t
                t = xp_t[img, k]
                src = xp_d.ap()[img, k].rearrange("p (h w) -> p h w", h=HPR)
                nc.sync.dma_start(t[:, r0:r1, :], src[:, r0:r1, :])

            def load_wt(lo, hi):
                nc.sync.dma_start(
                    wt_t[:, lo * 128 : hi * 128], wt_d.ap()[:, lo * 128 : hi * 128]
                )

            # minimal working set first: group order per macro is
            # A0B0lo(blk 0-11), A1B0lo(18-29), A0B1(12-17), A1B1(30-35),
            # A0B0hi, A1B0hi; first groups touch xp rows 0:6, B1 rows 0:10.
            load_xp_rows(0, 0, 0, 6)
            load_wt(0, 12)
            load_xp_rows(0, 1, 0, 6)
            nc.sync.dma_start(b_t[:], b_d.ap()[:])
            nc.vector.memset(zero_t[:], 0.0)
            load_wt(18, 30)
            load_xp_rows(0, 0, 6, 18)
            load_xp_rows(0, 1, 6, 18)
            load_wt(12, 18)
            load_wt(30, 36)
            load_xp_rows(0, 0, 18, HPR)
            load_xp_rows(0, 1, 18, HPR)
            load_wt(36, 54)
            load_wt(54, 72)
            load_xp_rows(1, 0, 0, HPR)
            load_xp_rows(1, 1, 0, HPR)

            zero_v = zero_t[:].rearrange("p (u a) -> p u a", a=2)
            for img in range(IMG_PER_CORE):
                for m in range(NM):
                    for mb in range(MB):
                        U0 = mb * MROWS
                        z_sb = zpool.tile([128, 2 * MROWS, ZW], dt.float32)
                        z_v = z_sb[:].rearrange(
                            "p (u a) (v b) -> p u a v b", a=2, b=2
                        )
                        # structural-zero pad cols cs=0 / cs=130 get delta
                        nc.scalar.activation(
                            z_v[:, :, :, 0, 0],
                            zero_v,
                            mybir.ActivationFunctionType.Identity,
                            bias=b_t[:, m : m + 1],
                        )
                        nc.scalar.activation(
                            z_v[:, :, :, 65, 0],
                            zero_v,
                            mybir.ActivationFunctionType.Identity,
                            bias=b_t[:, m : m + 1],
                        )

                        def mm_group(A, B, u0, rows, dst):
                            ncol = NCOL[B]
                            acc = psum_pool.tile([128, rows, ncol], dt.float32)
                            ntot = 2 * 3 * (2 - B)
                            n = 0
                            for k in range(NK):
                                for ti in range(3):
                                    for tj in range(2 - B):
                                        du, dv = DU[ti], DV[B][tj]
                                        idx = _blk(m, A, B, k, ti, tj)
                                        rhs = xp_t[img, k][
                                            :,
                                            u0 + du + 1 : u0 + du + 1 + rows,
                                            dv + 1 : dv + 1 + ncol,
                                        ]
                                        nc.tensor.matmul(
                                            acc[:],
                                            wt_t[:, idx * 128 : (idx + 1) * 128],
                                            rhs,
                                            start=(n == 0),
                                            stop=(n == ntot - 1),
                                        )
                                        n += 1
                            nc.scalar.activation(
                                dst,
                                acc[:],
                                mybir.ActivationFunctionType.Identity,
                                bias=b_t[:, m : m + 1],
                            )

                        def col_fir(half):
                            r0 = half * MROWS  # z/out row offset within macro
                            s1 = spool.tile([128, MROWS, 2 * W], dt.float32)
                            s2 = spool.tile([128, MROWS, 2 * W], dt.float32)
                            o_sb = opool.tile([128, MROWS, 2 * W], dt.float32)
                            zs = z_sb[:, r0 : r0 + MROWS, :]
                            nc.gpsimd.tensor_add(
                                s1[:], zs[:, :, 0:128], zs[:, :, 3:131]
                            )
                            nc.vector.tensor_add(
                                s2[:], zs[:, :, 1:129], zs[:, :, 2:130]
                            )
                            nc.vector.scalar_tensor_tensor(
                                o_sb[:],
                                s2[:],
                                3.0,
                                s1[:],
                                mybir.AluOpType.mult,
                                mybir.AluOpType.add,
                            )
                            nc.sync.dma_start(
                                out_d.ap()[
                                    img,
                                    m * 128 : (m + 1) * 128,
                                    16 * mb + r0 : 16 * mb + r0 + MROWS,
                                    :,
                                ],
                                o_sb[:],
                            )

                        # first half's rows (u 0:4, both A) complete after the
                        # two B0lo groups + both full-height B1 groups.
                        mm_group(0, 0, U0, 4, z_v[:, 0:4, 0, 0:66, 1])
                        mm_group(1, 0, U0, 4, z_v[:, 0:4, 1, 0:66, 1])
                        mm_group(0, 1, U0, 8, z_v[:, 0:8, 0, 1:65, 0])
                        mm_group(1, 1, U0, 8, z_v[:, 0:8, 1, 1:65, 0])
                        col_fir(0)
                        mm_group(0, 0, U0 + 4, 4, z_v[:, 4:8, 0, 0:66, 1])
                        mm_group(1, 0, U0 + 4, 4, z_v[:, 4:8, 1, 0:66, 1])
                        col_fir(1)

    nc.compile()
    return nc


def _compose_weights(w):
    """w (256,256,3,3) -> [128 ch_local, 72*128] bf16, row-FIR folded and
    the col-FIR 1/4 gain pre-applied."""
    g = np.array([1.0, 3.0, 3.0, 1.0], dtype=np.float64) / 4.0
    w64 = w.astype(np.float64)
    Wr = np.zeros((O, C, 6, 3), dtype=np.float64)
    for s in range(4):
        Wr[:, :, s : s + 3, :] += g[s] * w64
    Wpe = 0.25 * Wr
    blocks = np.empty((NBLK, 128, 128), dtype=np.float32)  # [blk, ch_l, o_l]
    for m in range(NM):
        for A in range(2):
            for B in range(2):
                for k in range(NK):
                    for ti in range(3):
                        for tj in range(2 - B):
                            sub = Wpe[
                                m * 128 : (m + 1) * 128,
                                k * 128 : (k + 1) * 128,
                                PL[A][ti],
                                JL[B][tj],
                            ]
                            blocks[_blk(m, A, B, k, ti, tj)] = sub.T.astype(
                                np.float32
                            )
    return np.ascontiguousarray(blocks.transpose(1, 0, 2)).reshape(128, -1)


def kernel(x, w, b):
    global _compiled, LAST_RESULTS
    if _compiled is None:
        _compiled = _build()
    nc = _compiled

    x = np.asarray(x, dtype=np.float32)
    w = np.asarray(w, dtype=np.float32)
    b = np.asarray(b, dtype=np.float32)

    wt = _compose_weights(w).astype(ml_dtypes.bfloat16)
    b2 = np.ascontiguousarray(b.reshape(NM, 128).T.astype(np.float32)) / 8.0
    xp = np.pad(x, ((0, 0), (0, 0), (1, 1), (1, 2)))  # (16, 256, 66, 67)
    xp = np.ascontiguousarray(
        xp.reshape(N_CORES, IMG_PER_CORE, NK, 128, HPR * WPC)
    ).astype(ml_dtypes.bfloat16)

    in_maps = [{"xp": xp[core], "wt": wt, "bias": b2} for core in range(N_CORES)]
    try:
        res = run_bass_kernel_spmd(nc, in_maps, list(range(N_CORES)))
    except ModuleNotFoundError:
        # BASS_TRACE set in an env without the axon NTFF hook module —
        # retry with tracing disabled.
        import os

        os.environ["BASS_NEVER_TRACE"] = "1"
        res = run_bass_kernel_spmd(nc, in_maps, list(range(N_CORES)))
    LAST_RESULTS = res
    out = np.concatenate([res.results[i]["out"] for i in range(N_CORES)], axis=0)
    return out


# revision 9
# speedup vs baseline: 1.9326x; 1.0467x over previous
"""StyleGAN2 fused upsample2x + 3x3 conv + FIR(1,3,3,1) + bias — TRN2 Bass kernel.

v4 — split the separable FIR: the ROW pass is folded into the conv weights,
the COLUMN pass runs on the vector engines.  Math:

  y1 = corr(zero-dilate-2(x), w, full pad)            # (129,129) per channel
  z[r,c'] = sum_s g[s] y1[r-1+s, c']   g=(1,3,3,1)/4  # row FIR -> 128 rows
  out[R,C] = sum_t g[t] z[R, C-1+t] + bias            # col FIR -> 128 cols

Folding the row FIR into w gives Wr (6 rows x 3 cols).  With output row
parity A = r%2 only 3 row taps survive (p in (1,3,5) for A=0, (0,2,4) for
A=1, all at u-offsets (-1,0,1)); with column parity B = c'%2 the zero
dilation leaves 2 col taps (B=0) or 1 (B=1): 18 matmul taps per output
pixel instead of 36 — half the PE work of composing the full 6x6 kernel.

Matmuls run in bf16 (inputs rounded host-side): same 1 cycle/row as
float32r but the self-loading LDWEIGHTS phase drops to 1 cycle/row so it
fully hides behind the 264-row moving pass, and input DMA bytes halve.
PSUM accumulation stays fp32; error lands ~5e-3, well inside the 2e-2 gate.

The 4-tap column FIR is symmetric: with the 1/4 gain folded into the
weights it is 3 fused element-wise ops: s1 = z[C]+z[C+3], s2 = z[C+1]+z[C+2],
out = 3*s2 + s1.  Pool (GpSimd) computes s1 (~2.2cyc/elem for
tensor_tensor), DVE computes s2 and the fused multiply-add (TensorScalarPtr
is DVE-only).  Bias is folded as delta = bias/8 on every stored z column
(including the two structural-zero pad columns): integer col taps sum to 8.

Tiling: o=256 in 2x128 (M), contraction 256 in 2x128 (K), spatial as 8
macro blocks of 8 U-rows (16 output rows).  Per macro and parity A: B=0 is
two 4-row PSUM groups [128,4,66] (264 free; PSUM bank = 512 fp32), B=1 one
8-row group [128,8,64] (512 free).  Groups are ordered so the first 8
output rows complete early and the column FIR + store run in two 8-row
halves overlapped with the PE.  Data-parallel: 2 images/core, 8 cores.
"""

import sys

sys.path.insert(0, "/opt/trn_rl_repo")

import ml_dtypes
import numpy as np

import concourse.bacc as bacc
import concourse.mybir as mybir
import concourse.tile as tile
from concourse.bass_utils import run_bass_kernel_spmd

N_CORES = 8
IMGS = 16
IMG_PER_CORE = IMGS // N_CORES  # 2
C = 256  # in channels
O = 256  # out channels
H = W = 64
NK = C // 128  # 2 contraction splits
NM = O // 128  # 2 output-partition splits
HPR = H + 2  # 66: padded input rows (1 halo each side)
WPC = H + 3  # 67: padded input cols (1 left, 2 right)
MB = 8  # macro row blocks
MROWS = 8  # U rows per macro block (16 output rows)
ZW = 132  # stored z width: cs=0..130 used (cs=c'+1, pads at 0 and 130)
NBLK = NM * 18 * NK  # 72 weight blocks of 128x128

PL = [(1, 3, 5), (0, 2, 4)]  # Wr row index by (A, ti)
JL = [(0, 2), (1,)]  # Wr col index by (B, tj)
DU = (-1, 0, 1)  # x row offset by ti (same for both A)
DV = [(-1, 0), (0,)]  # x col offset by (B, tj)
NCOL = (66, 64)  # z cols per (B); B=0 col 65 is pad-driven slack (cs=131)


def _blk(m, A, B, k, ti, tj):
    base = m * 36 + A * 18 + (12 if B else 0)
    return base + (k * 3 + ti) * (2 - B) + tj


_compiled = None
LAST_RESULTS = None


def _build():
    nc = bacc.Bacc(None, target_bir_lowering=False, debug=False)
    dt = mybir.dt

    xp_d = nc.dram_tensor(
        "xp", (IMG_PER_CORE, NK, 128, HPR * WPC), dt.bfloat16, kind="ExternalInput"
    )
    wt_d = nc.dram_tensor("wt", (128, NBLK * 128), dt.bfloat16, kind="ExternalInput")
    b_d = nc.dram_tensor("bias", (128, NM), dt.float32, kind="ExternalInput")
    out_d = nc.dram_tensor(
        "out", (IMG_PER_CORE, O, 2 * H, 2 * W), dt.float32, kind="ExternalOutput"
    )

    with tile.TileContext(nc) as tc:
        with (
            tc.tile_pool(name="xpool", bufs=1) as xpool,
            tc.tile_pool(name="wpool", bufs=1) as wpool,
            tc.tile_pool(name="zpool", bufs=6) as zpool,
            tc.tile_pool(name="spool", bufs=6) as spool,
            tc.tile_pool(name="opool", bufs=8) as opool,
            tc.tile_pool(name="psum", bufs=8, space="PSUM") as psum_pool,
        ):
            wt_t = wpool.tile([128, NBLK * 128], dt.bfloat16, tag="wt")
            b_t = wpool.tile([128, NM], dt.float32, tag="bias")
            zero_t = wpool.tile([128, 2 * MROWS], dt.float32, tag="zero")
            xp_t = {}

            def load_xp_rows(img, k, r0, r1):
                if (img, k) not in xp_t:
                    t = xpool.tile([128, HPR, WPC], dt.bfloat16, tag=f"xp{img}{k}")
                    xp_t[img, k] = t
                t = xp_t[img, k]
                src = xp_d.ap()[img, k].rearrange("p (h w) -> p h w", h=HPR)
                nc.sync.dma_start(t[:, r0:r1, :], src[:, r0:r1, :])

            def load_wt(lo, hi):
                nc.sync.dma_start(
                    wt_t[:, lo * 128 : hi * 128], wt_d.ap()[:, lo * 128 : hi * 128]
                )

            # minimal working set first: group order per macro is
            # A0B0lo(blk 0-11), A1B0lo(18-29), A0B1(12-17), A1B1(30-35),
            # A0B0hi, A1B0hi; first groups touch xp rows 0:6, B1 rows 0:10.
            load_xp_rows(0, 0, 0, 6)
            load_wt(0, 12)
            load_xp_rows(0, 1, 0, 6)
            nc.sync.dma_start(b_t[:], b_d.ap()[:])
            nc.vector.memset(zero_t[:], 0.0)
            load_wt(18, 30)
            load_xp_rows(0, 0, 6, 18)
            load_xp_rows(0, 1, 6, 18)
            load_wt(12, 18)
            load_wt(30, 36)
            load_xp_rows(0, 0, 18, HPR)
            load_xp_rows(0, 1, 18, HPR)
            load_wt(36, 54)
            load_wt(54, 72)
            load_xp_rows(1, 0, 0, HPR)
            load_xp_rows(1, 1, 0, HPR)

            zero_v = zero_t[:].rearrange("p (u a) -> p u a", a=2)
            for img in range(IMG_PER_CORE):
                for m in range(NM):
                    for mb in range(MB):
                        U0 = mb * MROWS
                        z_sb = zpool.tile([128, 2 * MROWS, ZW], dt.float32)
                        z_v = z_sb[:].rearrange(
                            "p (u a) (v b) -> p u a v b", a=2, b=2
                        )
                        # structural-zero pad cols cs=0 / cs=130 get delta
                        nc.scalar.activation(
                            z_v[:, :, :, 0, 0],
                            zero_v,
                            mybir.ActivationFunctionType.Identity,
                            bias=b_t[:, m : m + 1],
                        )
                        nc.scalar.activation(
                            z_v[:, :, :, 65, 0],
                            zero_v,
                            mybir.ActivationFunctionType.Identity,
                            bias=b_t[:, m : m + 1],
                        )

                        def mm_group(A, B, u0, rows, dst):
                            ncol = NCOL[B]
                            acc = psum_pool.tile([128, rows, ncol], dt.float32)
                            ntot = 2 * 3 * (2 - B)
                            n = 0
                            for k in range(NK):
                                for ti in range(3):
                                    for tj in range(2 - B):
                                        du, dv = DU[ti], DV[B][tj]
                                        idx = _blk(m, A, B, k, ti, tj)
                                        rhs = xp_t[img, k][
                                            :,
                                            u0 + du + 1 : u0 + du + 1 + rows,
                                            dv + 1 : dv + 1 + ncol,
                                        ]
                                        nc.tensor.matmul(
                                            acc[:],
                                            wt_t[:, idx * 128 : (idx + 1) * 128],
                                            rhs,
                                            start=(n == 0),
                                            stop=(n == ntot - 1),
                                        )
                                        n += 1
                            nc.scalar.activation(
                                dst,
                                acc[:],
                                mybir.ActivationFunctionType.Identity,
                                bias=b_t[:, m : m + 1],
                            )

                        def col_fir(half):
                            r0 = half * MROWS  # z/out row offset within macro
                            s1 = spool.tile([128, MROWS, 2 * W], dt.float32)
                            s2 = spool.tile([128, MROWS, 2 * W], dt.float32)
                            o_sb = opool.tile([128, MROWS, 2 * W], dt.float32)
                            zs = z_sb[:, r0 : r0 + MROWS, :]
                            nc.gpsimd.tensor_add(
                                s1[:], zs[:, :, 0:128], zs[:, :, 3:131]
                            )
                            nc.vector.tensor_add(
                                s2[:], zs[:, :, 1:129], zs[:, :, 2:130]
                            )
                            nc.vector.scalar_tensor_tensor(
                                o_sb[:],
                                s2[:],
                                3.0,
                                s1[:],
                                mybir.AluOpType.mult,
                                mybir.AluOpType.add,
                            )
                            nc.sync.dma_start(
                                out_d.ap()[
                                    img,
                                    m * 128 : (m + 1) * 128,
                                    16 * mb + r0 : 16 * mb + r0 + MROWS,
                                    :,
                                ],
                                o_sb[:],
                            )

                        # first half's rows (u 0:4, both A) complete after the
                        # two B0lo groups + both full-height B1 groups.
                        # 8 uniform 4-row groups per macro (one PSUM bank
                        # each, ring-aligned with the 8 banks); each 8-row
                        # half completes after 4 groups, then its column FIR
                        # + store overlap the next 4.
                        mm_group(0, 0, U0, 4, z_v[:, 0:4, 0, 0:66, 1])
                        mm_group(1, 0, U0, 4, z_v[:, 0:4, 1, 0:66, 1])
                        mm_group(0, 1, U0, 4, z_v[:, 0:4, 0, 1:65, 0])
                        mm_group(1, 1, U0, 4, z_v[:, 0:4, 1, 1:65, 0])
                        col_fir(0)
                        mm_group(0, 0, U0 + 4, 4, z_v[:, 4:8, 0, 0:66, 1])
                        mm_group(1, 0, U0 + 4, 4, z_v[:, 4:8, 1, 0:66, 1])
                        mm_group(0, 1, U0 + 4, 4, z_v[:, 4:8, 0, 1:65, 0])
                        mm_group(1, 1, U0 + 4, 4, z_v[:, 4:8, 1, 1:65, 0])
                        col_fir(1)

    nc.compile()
    return nc


def _compose_weights(w):
    """w (256,256,3,3) -> [128 ch_local, 72*128] bf16, row-FIR folded and
    the col-FIR 1/4 gain pre-applied."""
    g = np.array([1.0, 3.0, 3.0, 1.0], dtype=np.float64) / 4.0
    w64 = w.astype(np.float64)
    Wr = np.zeros((O, C, 6, 3), dtype=np.float64)
    for s in range(4):
        Wr[:, :, s : s + 3, :] += g[s] * w64
    Wpe = 0.25 * Wr
    blocks = np.empty((NBLK, 128, 128), dtype=np.float32)  # [blk, ch_l, o_l]
    for m in range(NM):
        for A in range(2):
            for B in range(2):
                for k in range(NK):
                    for ti in range(3):
                        for tj in range(2 - B):
                            sub = Wpe[
                                m * 128 : (m + 1) * 128,
                                k * 128 : (k + 1) * 128,
                                PL[A][ti],
                                JL[B][tj],
                            ]
                            blocks[_blk(m, A, B, k, ti, tj)] = sub.T.astype(
                                np.float32
                            )
    return np.ascontiguousarray(blocks.transpose(1, 0, 2)).reshape(128, -1)


def kernel(x, w, b):
    global _compiled, LAST_RESULTS
    if _compiled is None:
        _compiled = _build()
    nc = _compiled

    x = np.asarray(x, dtype=np.float32)
    w = np.asarray(w, dtype=np.float32)
    b = np.asarray(b, dtype=np.float32)

    wt = _compose_weights(w).astype(ml_dtypes.bfloat16)
    b2 = np.ascontiguousarray(b.reshape(NM, 128).T.astype(np.float32)) / 8.0
    xp = np.pad(x, ((0, 0), (0, 0), (1, 1), (1, 2)))  # (16, 256, 66, 67)
    xp = np.ascontiguousarray(
        xp.reshape(N_CORES, IMG_PER_CORE, NK, 128, HPR * WPC)
    ).astype(ml_dtypes.bfloat16)

    in_maps = [{"xp": xp[core], "wt": wt, "bias": b2} for core in range(N_CORES)]
    try:
        res = run_bass_kernel_spmd(nc, in_maps, list(range(N_CORES)))
    except ModuleNotFoundError:
        # BASS_TRACE set in an env without the axon NTFF hook module —
        # retry with tracing disabled.
        import os

        os.environ["BASS_NEVER_TRACE"] = "1"
        res = run_bass_kernel_spmd(nc, in_maps, list(range(N_CORES)))
    LAST_RESULTS = res
    out = np.concatenate([res.results[i]["out"] for i in range(N_CORES)], axis=0)
    return out
